# revision 31
# baseline (speedup 1.0000x reference)
"""2-layer GCN (DGL GraphConv norm='both') on 8 Trainium2 NeuronCores.

Strategy (graph/data parallel, dst-sharded):
  - Host: degree norms, pre-scale+transpose features to bf16, sort each
    core's edges by (dst-window, src-half, src), pack into 128-edge tiles.
  - Device per core c (nodes [c*6250, (c+1)*6250)):
      h_sh = XnT_sh.T @ W1            (dense matmuls, bf16, f32 psum)
      AllGather h -> full h table [50176, 128] bf16 in HBM
      agg1 = segment_sum(h[src], dst) via dma_gather (rows->partitions)
             + onehot matmuls accumulating in PSUM per 64-dst window
      x1n = relu(agg1 * (ndst*nsrc))  (b1 == 0 fast path; fused DVE op)
      y_sh = x1n_sh @ W2  per window-pair (PE transpose + matmul)  [NSHP, 8]
      AllGather y -> full y table [50176, 128] bf16 (payload cols 0:8)
      agg2 = segment_sum(y[src], dst) via 16B-payload dma_gather (7ns/desc
             floor) + onehot matmuls into [64, 8] PSUM per window
      out = agg2 * ndst -> [NSHP, 8] f32
  - Host: slice [:, :7], add b2, concat cores.

Layer-2 aggregates in the 8-dim output space (aggregation commutes with
the linear W2), so its gather payload is 16 B/edge instead of 256 B.  The
per-edge gather descriptor hits the DMA cost floor; the y table keeps a
256 B row pitch (gather stride must be a multiple of 256 B) with payload
in cols 0:8.

Edge tiles: 128 edges, one 64-dst window, one src half (lo: table rows
< 32768, hi: >= 32768 -- dma_gather indices are int16).  The per-window
per-half tile counts are max'd across cores so all 8 cores share one
program; short cores pad with (src=0, localdst=200) no-op edges.
"""

import numpy as np
import ml_dtypes

import concourse.bass as bass
import concourse.bacc as bacc
import concourse.mybir as mybir
import concourse.tile as tile
from concourse import bass_utils
from concourse._compat import exact_div

BF16 = ml_dtypes.bfloat16

N = 50000
E = 1600000
FIN = 1433
FP = 1536            # FIN padded to 12*128
H = 128
C = 7
NCORES = 8
NSH = N // NCORES    # 6250
W = 64               # dst window width
NW = (NSH + W - 1) // W          # 98 windows per core
NSHP = NW * W                    # 6272 padded shard rows
TROWS = NCORES * NSHP            # 50176 table rows
LO = 32768                       # lo/hi table split
H8 = True            # layer-1 h table in fp8e4m3 (halves gather DMA time)
HROW = 256 if H8 else 128        # h table row pitch in elements (256B)
BT = 64 if H8 else 48            # tiles per dma_gather call (layer 1)
BT2 = 96             # tiles per gather call (layer 2, small payload)
PAD_DST = 200.0      # local-dst sentinel for pad edges (> W-1)

KB = NW * W // 128   # 49 node blocks of 128 in dense stage
NQ = KB              # 49 window pairs
KCH = FP // 128      # 12 contraction chunks
GP_BUFS = 3          # gather-tile buffering
OH_BUFS = 3          # onehot-tile buffering
PSW_BUFS = 3         # window psum buffering
NSB = 4              # node blocks per xnt load (512-wide psum bank)
NSWQ = 1             # SWDGE queues


def _ceil_div(a, b):
    return (a + b - 1) // b


def _wrap_idx(idx_flat):
    """[T*128] -> [128, T*8] int16: position i -> [i%16 (+16k copies), i//16]."""
    a = np.asarray(idx_flat, np.int16).reshape(-1, 16).T  # [16, T*8]
    return np.ascontiguousarray(np.tile(a, (8, 1)))       # [128, T*8]


def dma_gather_raw(gp, out_ap, in_ap, idxs_ap, num_idxs, num_idxs_reg,
                   elem_size, elem_step, queue_num=0, single_packet=False):
    """nc.gpsimd.dma_gather (non-transpose, HBM source) without the
    elem_size%256B assert; payload (elem_size) < row pitch (elem_step) is
    supported by the ucode (stride_bytes_256 is a separate field) and was
    verified bit-exact on hardware."""
    stride_bytes = elem_step * mybir.dt.size(in_ap.dtype)
    stride_bytes_256 = exact_div(stride_bytes, 256)
    assert stride_bytes_256 < 256
    _in_ap = gp.lower_ap_dma(in_ap, for_custom_bir_dma=True)
    _idxs_ap = gp.lower_ap(idxs_ap)
    _out_ap = gp.lower_ap(out_ap)
    return gp.add_instruction(
        mybir.InstDMAGatherAnt(
            name=gp.bass.get_next_instruction_name(),
            ins=[*_in_ap, _idxs_ap,
                 gp.lower_val_access(gp.to_reg(num_idxs_reg))],
            outs=[_out_ap],
            transpose=False,
            num_idxs=num_idxs,
            elem_size=elem_size,
            stride_bytes_256=stride_bytes_256,
            gen_mode=0,
            single_packet=single_packet,
            queue_num=queue_num,
            sbuf_tokens_per_rank=0,
            sbuf_free_dim_per_rank=0,
            sbuf_free_dim_pad_per_rank=0,
            sbuf_byte_offset=0,
        )
    )


def _balance_positions(src, dst):
    """Assign nodes to padded table positions so every (core, window) bucket's
    lo-src edge count packs close under a shared per-window tile budget.
    Removes most of the ceil(max-over-cores/128) tile padding.  Node halves
    (table row < LO) are fixed up front so src half membership is stable.
    Returns (pos[node] -> position, node_at[position] -> node or -1)."""
    import heapq

    nodes = np.arange(N)
    c0 = nodes // NSH
    p0 = c0 * NSHP + (nodes - c0 * NSH)
    is_lo_node = p0 < LO
    src_lo = is_lo_node[src]
    dlo = np.bincount(dst[src_lo], minlength=N).astype(np.int64)

    # bucket (c, w) is lo-class iff its 64 positions all lie below LO
    bc = np.repeat(np.arange(NCORES), NW)
    bw = np.tile(np.arange(NW), NCORES)
    b_lo = (bc * NSHP + (bw + 1) * W) <= LO

    # per-window lo-edge capacity 128*t, t in {tl, tl+1}.  Total capacity is
    # kept tight (~0.4% slack) so the max-remaining-capacity greedy packs
    # fills just under each window's 128-tile boundary instead of smearing
    # slack across all windows.
    core_load = dlo.sum() / NCORES
    tl = int(core_load // (NW * 128))
    band = 48   # guard band: endgame slot-filling overshoots caps by ~20-40
    nplus = int(np.clip(np.ceil((core_load + NW * band - NW * tl * 128)
                                / 128), 1, NW))
    t_lo = np.full(NW, tl, np.int64)
    t_lo[:nplus] += 1
    cap = (t_lo[bw] * 128 - band).astype(np.int64)

    slots = np.full(NCORES * NW, W, np.int64)
    heaps = {True: [], False: []}
    for b in range(NCORES * NW):
        heaps[bool(b_lo[b])].append((-cap[b], b))
    heapq.heapify(heaps[True])
    heapq.heapify(heaps[False])

    order = np.argsort(-dlo, kind="stable")
    assign = np.empty(N, np.int64)
    for n in order:
        h = heaps[bool(is_lo_node[n])]
        skipped = []
        while True:
            negcap, b = heapq.heappop(h)
            if slots[b] > 0:
                break
            skipped.append((negcap, b))
        for e in skipped:
            heapq.heappush(h, e)
        assign[n] = b
        slots[b] -= 1
        if slots[b] > 0:
            heapq.heappush(h, (negcap + dlo[n], b))

    pos = np.empty(N, np.int64)
    node_at = np.full(TROWS, -1, np.int64)
    fill = np.zeros(NCORES * NW, np.int64)
    for n in range(N):
        b = assign[n]
        p = bc[b] * NSHP + bw[b] * W + fill[b]
        fill[b] += 1
        pos[n] = p
        node_at[p] = n
    return pos, node_at


def _prep(features, src, dst, W1, b1, W2, b2):
    """Host-side sharding/packing. Returns (in_maps, schedule info)."""
    src = np.asarray(src).astype(np.int64)
    dst = np.asarray(dst).astype(np.int64)
    features = np.asarray(features, np.float32)
    b1f = np.asarray(b1, np.float32)
    fuse_b1 = bool(np.all(b1f == 0.0))

    deg_src = np.bincount(src, minlength=N).astype(np.float32)
    deg_dst = np.bincount(dst, minlength=N).astype(np.float32)
    nsrc = 1.0 / np.sqrt(np.maximum(deg_src, 1.0))
    ndst = 1.0 / np.sqrt(np.maximum(deg_dst, 1.0))

    pos, node_at = _balance_positions(src, dst)

    # padded global table ids
    g_src = pos[src]

    # per-core edge groups
    p_dst = pos[dst]
    dcore = p_dst // NSHP
    dloc = p_dst - dcore * NSHP
    win = dloc // W
    half = (g_src >= LO).astype(np.int64)

    cnt = np.zeros((NCORES, NW, 2), np.int64)
    per_core = []
    for c in range(NCORES):
        m = dcore == c
        gs, wn, hf, dl = g_src[m], win[m], half[m], dloc[m]
        order = np.lexsort((gs, hf, wn))
        gs, wn, hf, dl = gs[order], wn[order], hf[order], dl[order]
        key = wn * 2 + hf
        cnt[c] = np.bincount(key, minlength=NW * 2).reshape(NW, 2)
        per_core.append((gs, dl, key))

    tw = np.zeros((NW, 2), np.int64)  # shared schedule: tiles per (win, half)
    for h in range(2):
        tw[:, h] = _ceil_div(np.max(cnt[:, :, h], axis=0), 128)

    # tile slots: pass-lo tiles for w=0..NW-1, then pass-hi
    tile_win = []
    tile_base = np.zeros((NW, 2), np.int64)
    for h in range(2):
        for w in range(NW):
            tile_base[w, h] = len(tile_win)
            tile_win.extend([w] * int(tw[w, h]))
    T = len(tile_win)

    # dense-stage feature prep (shared)
    Xn = features * nsrc[:, None]

    w1p = np.zeros((FP, H), np.float32)
    w1p[:FIN] = W1
    w1p = w1p.astype(BF16)
    w2p = np.zeros((H, 8), np.float32)
    w2p[:, :C] = W2
    w2p = w2p.astype(BF16)
    # iota repeated BT2 times per value, w-major: col w*BT2+n -> w.  Lets the
    # onehot is_equal keep packed last dims on every operand (DVE 2x mode).
    iotar = np.repeat(np.arange(W, dtype=np.float32), BT2)
    iotar = np.tile(iotar, (128, 1)).astype(BF16)
    ident = np.vstack([np.eye(W, dtype=np.float32)] * 2)        # [128, 64]
    identb = np.eye(128, dtype=np.float32).astype(BF16)         # [128, 128]
    b1rep = np.tile(b1f, (2 * W, 1))                            # [128, 128]

    in_maps = []
    for c in range(NCORES):
        gs, dl, key = per_core[c]
        idx_flat = np.zeros(T * 128, np.int64)
        ldst_flat = np.full(T * 128, PAD_DST, np.float32)
        starts = np.zeros(NW * 2 + 1, np.int64)
        starts[1:] = np.cumsum(np.bincount(key, minlength=NW * 2))
        for h in range(2):
            for w in range(NW):
                k = w * 2 + h
                n = starts[k + 1] - starts[k]
                if n == 0:
                    continue
                slot = tile_base[w, h] * 128
                idx_flat[slot:slot + n] = gs[starts[k]:starts[k + 1]] - h * LO
                ldst_flat[slot:slot + n] = dl[starts[k]:starts[k + 1]] % W

        cslice = node_at[c * NSHP:(c + 1) * NSHP]
        valid = cslice >= 0
        xnt = np.zeros((FP, NSHP), np.float32)
        xnt[:FIN, valid] = Xn[cslice[valid]].T

        pad_d = np.zeros(NSHP, np.float32)
        pad_d[valid] = ndst[cslice[valid]]
        nsd = pad_d.reshape(NW, W).T
        nsd = np.vstack([nsd, nsd])                             # [128, NW]
        pad_s = np.zeros(NSHP, np.float32)
        pad_s[valid] = nsrc[cslice[valid]]
        nss = pad_s.reshape(NW, W).T
        nss = np.vstack([nss, nss])

        im = {
            "xnt": xnt.astype(BF16),
            "w1": w1p,
            "w2": w2p,
            "iota": iotar,
            "identf": ident,
            "identb": identb,
            "nsd": np.ascontiguousarray(nsd),
            "idx": _wrap_idx(idx_flat),
            "ldst": np.ascontiguousarray(
                ldst_flat.reshape(T, 128).T).astype(BF16),
        }
        if fuse_b1:
            im["nsdss"] = np.ascontiguousarray(nsd * nss)
        else:
            im["nss"] = np.ascontiguousarray(nss)
            im["b1rep"] = b1rep
        in_maps.append(im)
    return in_maps, tw, tile_win, tile_base, T, fuse_b1, node_at


def _build_program(tw, tile_win, T, timing=False, fuse_b1=True):
    nc = bacc.Bacc("TRN2", target_bir_lowering=False, debug=False,
                   num_devices=NCORES, num_swdge_queues=NSWQ)
    dt = mybir.dt
    xnt_d = nc.dram_tensor("xnt", [FP, NSHP], dt.bfloat16, kind="ExternalInput")
    w1_d = nc.dram_tensor("w1", [FP, H], dt.bfloat16, kind="ExternalInput")
    w2_d = nc.dram_tensor("w2", [H, 8], dt.bfloat16, kind="ExternalInput")
    iota_d = nc.dram_tensor("iota", [128, W * BT2], dt.bfloat16,
                            kind="ExternalInput")
    identf_d = nc.dram_tensor("identf", [2 * W, W], dt.float32, kind="ExternalInput")
    identb_d = nc.dram_tensor("identb", [128, 128], dt.bfloat16,
                              kind="ExternalInput")
    nsd_d = nc.dram_tensor("nsd", [2 * W, NW], dt.float32, kind="ExternalInput")
    if fuse_b1:
        nsdss_d = nc.dram_tensor("nsdss", [2 * W, NW], dt.float32,
                                 kind="ExternalInput")
    else:
        nss_d = nc.dram_tensor("nss", [2 * W, NW], dt.float32,
                               kind="ExternalInput")
        b1_d = nc.dram_tensor("b1rep", [2 * W, H], dt.float32,
                              kind="ExternalInput")
    idx_d = nc.dram_tensor("idx", [128, T * 8], dt.int16, kind="ExternalInput")
    ldst_d = nc.dram_tensor("ldst", [128, T], dt.bfloat16, kind="ExternalInput")
    out_d = nc.dram_tensor("out", [NSHP, 8], dt.float32, kind="ExternalOutput")

    ntl = int(tw[:, 0].sum())

    with tile.TileContext(nc) as tc:
        with (
            tc.tile_pool(name="const", bufs=1) as cpool,
            tc.tile_pool(name="xnt", bufs=2) as xpool,
            tc.tile_pool(name="g", bufs=GP_BUFS) as gpool,
            tc.tile_pool(name="oh", bufs=OH_BUFS) as ohpool,
            tc.tile_pool(name="ep", bufs=2) as eppool,
            tc.tile_pool(name="small", bufs=2) as spool,
            tc.tile_pool(name="psA", bufs=2, space="PSUM") as psA,
            tc.tile_pool(name="psW", bufs=PSW_BUFS, space="PSUM") as psW,
            tc.tile_pool(name="psT", bufs=1, space="PSUM") as psT,
            tc.tile_pool(name="dram", bufs=1, space="DRAM") as dram,
        ):
            # ---- constants ----
            w1_sb = cpool.tile([128, KCH * H], dt.bfloat16, tag="w1")
            nc.sync.dma_start(
                w1_sb[:].rearrange("p (k h) -> p k h", h=H),
                w1_d[:].rearrange("(k p) h -> p k h", p=128))
            w2_sb = cpool.tile([128, 8], dt.bfloat16, tag="w2")
            nc.sync.dma_start(w2_sb[:], w2_d[:])
            iota_sb = cpool.tile([128, W * BT2], dt.bfloat16, tag="iota")
            nc.sync.dma_start(iota_sb[:], iota_d[:])
            identf_sb = cpool.tile([2 * W, W], dt.float32, tag="idf")
            nc.sync.dma_start(identf_sb[:], identf_d[:])
            identb_sb = cpool.tile([128, 128], dt.bfloat16, tag="idb")
            nc.sync.dma_start(identb_sb[:], identb_d[:])
            nsd_sb = cpool.tile([2 * W, NW], dt.float32, tag="nsd")
            nc.sync.dma_start(nsd_sb[:], nsd_d[:])
            if fuse_b1:
                nsdss_sb = cpool.tile([2 * W, NW], dt.float32, tag="nsdss")
                nc.sync.dma_start(nsdss_sb[:], nsdss_d[:])
            else:
                nss_sb = cpool.tile([2 * W, NW], dt.float32, tag="nss")
                nc.sync.dma_start(nss_sb[:], nss_d[:])
                b1_sb = cpool.tile([2 * W, H], dt.float32, tag="b1")
                nc.sync.dma_start(b1_sb[:], b1_d[:])
            idx_sb = cpool.tile([128, T * 8], dt.int16, tag="idx")
            nc.sync.dma_start(idx_sb[:], idx_d[:])
            ldst_sb = cpool.tile([128, T], dt.bfloat16, tag="ldst")
            nc.sync.dma_start(ldst_sb[:], ldst_d[:])
            # x1acc: window w -> partitions (w%2)*64..+64, cols (w//2)*128..+128
            x1acc = cpool.tile([128, NQ * H], dt.float32, tag="acc")
            x1stage = cpool.tile([128, NQ * H], dt.bfloat16, tag="xst")
            x2acc = cpool.tile([128, NQ * 8], dt.float32, tag="acc2")
            y_sb = cpool.tile([128, NQ * 8], dt.bfloat16, tag="ysb")
            out_sb = cpool.tile([128, NQ * 8], dt.float32, tag="out")

            hdt = dt.float8e4 if H8 else dt.bfloat16
            ag_h_in = dram.tile([NSHP, HROW], hdt)
            ag_y_in = dram.tile([NSHP, H], dt.bfloat16)
            if timing:
                h_full = dram.tile([TROWS, HROW], hdt)
                y_full = dram.tile([TROWS, H], dt.bfloat16)
            else:
                h_full = dram.tile([TROWS, HROW], hdt,
                                   addr_space="Shared")
                y_full = dram.tile([TROWS, H], dt.bfloat16,
                                   addr_space="Shared")

            # PE warm-up: ~4us of back-to-back tiny matmuls so the dense
            # stage starts at full clock (cost model p-state ramp).
            pwu = psT.tile([8, 8], dt.float32, tag="pwu")
            for _ in range(40):
                nc.tensor.matmul(out=pwu[:], lhsT=identb_sb[:, 0:8],
                                 rhs=identb_sb[:, 0:8], start=True, stop=True)

            # ---- stage B: h_sh = XnT_sh.T @ W1 ----
            # 8-block xnt loads alternate between the SP and DVE DMA queues
            # (per-call fixed cost ~2.5us would otherwise serialize); each
            # load feeds two 4-block matmul groups (512-wide psum bank).
            LG = 2 * NSB         # blocks per load
            li = 0
            for lb0 in range(0, KB, LG):
                nlb = min(LG, KB - lb0)
                xnt_sb = xpool.tile([128, KCH * LG * 128], dt.bfloat16,
                                    tag="xnt")
                eng = nc.sync if li % 2 == 0 else nc.scalar
                li += 1
                eng.dma_start(
                    xnt_sb[:, :KCH * nlb * 128].rearrange(
                        "p (k n) -> p k n", k=KCH),
                    xnt_d[:, lb0 * 128:(lb0 + nlb) * 128].rearrange(
                        "(k p) n -> p k n", p=128))
                hb = eppool.tile([128, LG * 128], hdt, tag="hb")
                for g in range(0, nlb, NSB):
                    nsb = min(NSB, nlb - g)
                    phw = psA.tile([128, NSB * 128], dt.float32, tag="ph")
                    for k in range(KCH):
                        nc.tensor.matmul(
                            out=phw[:, :nsb * 128],
                            lhsT=w1_sb[:, k * H:(k + 1) * H],
                            rhs=xnt_sb[:, (k * nlb + g) * 128:
                                       (k * nlb + g + nsb) * 128],
                            start=(k == 0), stop=(k == KCH - 1))
                    hsb = spool.tile([128, NSB * 128], dt.bfloat16, tag="hsb")
                    nc.scalar.copy(out=hsb[:, :nsb * 128],
                                   in_=phw[:, :nsb * 128])
                    for nb in range(nsb):
                        pt = psT.tile([128, H], dt.bfloat16, tag="pt")
                        nc.tensor.transpose(
                            out=pt[:], in_=hsb[:, nb * 128:(nb + 1) * 128],
                            identity=identb_sb[:])
                        nc.scalar.copy(
                            out=hb[:, (g + nb) * 128:(g + nb + 1) * 128],
                            in_=pt[:])
                hbase = ag_h_in[lb0 * 128:(lb0 + nlb) * 128, 0:H]
                nc.gpsimd.dma_start(
                    bass.AP(hbase.tensor, hbase.offset,
                            [[HROW, 128], [128 * HROW, nlb], [1, H]]),
                    hb[:, :nlb * 128].rearrange("p (b c) -> p b c", b=nlb))
                if timing:
                    nc.gpsimd.dma_start(
                        h_full[lb0 * 128:(lb0 + nlb) * 128, :],
                        ag_h_in[lb0 * 128:(lb0 + nlb) * 128, :])

            if timing:
                pass
            else:
                nc.gpsimd.collective_compute(
                    "AllGather", mybir.AluOpType.bypass,
                    replica_groups=[list(range(NCORES))],
                    ins=[ag_h_in[:].opt()], outs=[h_full[:].opt()])

            # ---- generic two-half aggregation over the edge tiles ----
            def aggregate(table, acc, accw, bt, elem, win_epilogue, win_copy,
                          win_done=None, bt_o=None, gdt=dt.bfloat16,
                          estep=H):
                bt_o = bt_o or bt
                nc.vector.memset(acc[:], 0.0)
                pw = {}      # window -> psum tile
                nmm = {}     # window -> matmuls issued this pass
                done = set()

                def finish(w, psum):
                    win_epilogue(w, psum)
                    done.add(w)
                    if win_done is None:
                        return
                    q = w // 2
                    if 2 * q in done and 2 * q + 1 in done:
                        win_done(q)

                for hf in range(2):
                    t0 = 0 if hf == 0 else ntl
                    t1 = ntl if hf == 0 else T
                    tbl = (table[0:LO, 0:elem] if hf == 0
                           else table[LO:TROWS, 0:elem])
                    if hf == 1:
                        # windows with zero hi tiles: seed + epilogue now so
                        # pair completion stays monotonic during the hi pass
                        for w in range(NW):
                            if tw[w, 1] == 0:
                                p = psW.tile([2 * W, accw], dt.float32,
                                             tag="pw")
                                po = (w % 2) * W
                                co = (w // 2) * accw
                                nc.tensor.matmul(
                                    out=p[po:po + W, :],
                                    lhsT=identf_sb[po:po + W, :],
                                    rhs=acc[po:po + W, co:co + accw],
                                    start=True, stop=True)
                                finish(w, p)
                    sched = []
                    b0 = t0
                    ramp = [max(bt // 3, 8), max(2 * bt // 3, 8)]
                    for r in ramp:
                        if t1 - b0 > bt:
                            sched.append((b0, r))
                            b0 += r
                    while t1 - b0 > bt:
                        sched.append((b0, bt))
                        b0 += bt
                    rem = t1 - b0
                    if rem > 2 * bt // 3:
                        sched.append((b0, 2 * rem // 3))
                        sched.append((b0 + 2 * rem // 3, rem - 2 * rem // 3))
                    elif rem > 0:
                        sched.append((b0, rem))
                    for b0, nt in sched:
                        gt = gpool.tile([128, bt * elem], gdt, tag="g")
                        dma_gather_raw(
                            nc.gpsimd,
                            out_ap=gt[:, :nt * elem].rearrange(
                                "p (n e) -> p n e", e=elem),
                            in_ap=tbl,
                            idxs_ap=idx_sb[:, b0 * 8:(b0 + nt) * 8],
                            num_idxs=nt * 128,
                            num_idxs_reg=nt * 128,
                            elem_size=elem,
                            elem_step=estep,
                            single_packet=False,
                            queue_num=0)
                        for o0 in range(0, nt, bt_o):
                            no = min(bt_o, nt - o0)
                            oh = ohpool.tile([128, bt_o * W], dt.bfloat16,
                                             tag="oh")
                            ld = ldst_sb[:, b0 + o0:b0 + o0 + no]
                            ohb = oh[:]
                            irb = iota_sb[:]
                            nc.vector.tensor_tensor(
                                out=bass.AP(ohb.tensor, ohb.offset,
                                            [ohb.ap[0], [bt_o, W], [1, no]]),
                                in0=bass.AP(ld.tensor, ld.offset,
                                            [ld.ap[0], [0, W], ld.ap[1]]),
                                in1=bass.AP(irb.tensor, irb.offset,
                                            [irb.ap[0], [BT2, W], [1, no]]),
                                op=mybir.AluOpType.is_equal)
                            for j in range(no):
                                t = b0 + o0 + j
                                w = tile_win[t]
                                po, co = (w % 2) * W, (w // 2) * accw
                                if w not in pw:
                                    pw[w] = psW.tile([2 * W, accw],
                                                     dt.float32,
                                                     name=f"pw{w}", tag="pw")
                                    nmm[w] = 0
                                    if hf == 1:  # seed from pass-lo partial
                                        nc.tensor.matmul(
                                            out=pw[w][po:po + W, :],
                                            lhsT=identf_sb[po:po + W, :],
                                            rhs=acc[po:po + W, co:co + accw],
                                            start=True, stop=False)
                                        nmm[w] = 1
                                last_of_win = (t + 1 == t1 or
                                               tile_win[t + 1] != w)
                                ohj = oh[:, j:j + 1]
                                nc.tensor.matmul(
                                    out=pw[w][po:po + W, :],
                                    lhsT=bass.AP(ohj.tensor, ohj.offset,
                                                 [ohj.ap[0], [bt_o, W]]),
                                    rhs=gt[:, (o0 + j) * elem:
                                           (o0 + j) * elem + accw],
                                    start=(nmm[w] == 0), stop=last_of_win)
                                nmm[w] += 1
                                if last_of_win:
                                    if hf == 0:
                                        win_copy(w, pw.pop(w))
                                    else:
                                        finish(w, pw.pop(w))
                assert not pw, f"unclosed windows {list(pw)}"

            # ---- layer 1 ----
            def copy1(w, psum):
                po, co = (w % 2) * W, (w // 2) * H
                nc.scalar.copy(out=x1acc[po:po + W, co:co + H],
                               in_=psum[po:po + W, :])

            def epilogue1(w, psum):
                po, co = (w % 2) * W, (w // 2) * H
                ps = psum[po:po + W, :]
                if fuse_b1:
                    nc.scalar.activation(
                        out=x1stage[po:po + W, co:co + H], in_=ps,
                        func=mybir.ActivationFunctionType.Relu,
                        bias=0.0, scale=nsdss_sb[po:po + W, w:w + 1])
                else:
                    u = eppool.tile([2 * W, H], dt.float32, tag="ep")
                    uh = u[po:po + W, :]
                    nc.vector.tensor_scalar(
                        out=uh, in0=ps, scalar1=nsd_sb[po:po + W, w:w + 1],
                        scalar2=None, op0=mybir.AluOpType.mult)
                    nc.vector.tensor_tensor(
                        out=uh, in0=uh, in1=b1_sb[po:po + W, :],
                        op=mybir.AluOpType.add)
                    nc.vector.tensor_scalar(
                        out=x1stage[po:po + W, co:co + H], in0=uh,
                        scalar1=nss_sb[po:po + W, w:w + 1], scalar2=0.0,
                        op0=mybir.AluOpType.mult, op1=mybir.AluOpType.max)

            # y = x1n @ W2 per window pair, pipelined into the L1 hi pass
            ydone = set()
            yhalf = [range(0, 25), range(25, NQ)]

            def ystage_flush(hi):
                q0, q1 = (0, 25) if hi == 0 else (25, NQ)
                ybase = ag_y_in[q0 * 128:q1 * 128, 0:8]
                nc.scalar.dma_start(
                    bass.AP(ybase.tensor, ybase.offset,
                            [[H, 128], [128 * H, q1 - q0], [1, 8]]),
                    y_sb[:, q0 * 8:q1 * 8].rearrange(
                        "p (q c) -> p q c", c=8))
                if timing:
                    nc.scalar.dma_start(
                        y_full[q0 * 128:q1 * 128, :],
                        ag_y_in[q0 * 128:q1 * 128, :])

            def ystage(q):
                pt = psT.tile([128, H], dt.bfloat16, tag="pt")
                nc.tensor.transpose(out=pt[:],
                                    in_=x1stage[:, q * H:(q + 1) * H],
                                    identity=identb_sb[:])
                at = spool.tile([128, H], dt.bfloat16, tag="at")
                nc.scalar.copy(out=at[:], in_=pt[:])
                py = psT.tile([128, 8], dt.float32, tag="py")
                nc.tensor.matmul(out=py[:], lhsT=at[:], rhs=w2_sb[:],
                                 start=True, stop=True)
                nc.scalar.copy(out=y_sb[:, q * 8:(q + 1) * 8], in_=py[:])
                ydone.add(q)
                for hi in range(2):
                    if all(x in ydone for x in yhalf[hi]) and \
                            ("f", hi) not in ydone:
                        ydone.add(("f", hi))
                        ystage_flush(hi)

            aggregate(h_full, x1acc, H, BT, H, epilogue1, copy1,
                      win_done=ystage, gdt=hdt, estep=HROW)
            if timing:
                pass
            else:
                nc.gpsimd.collective_compute(
                    "AllGather", mybir.AluOpType.bypass,
                    replica_groups=[list(range(NCORES))],
                    ins=[ag_y_in[:].opt()], outs=[y_full[:].opt()])

            # ---- layer 2 (aggregate y, 16B payload per edge) ----
            def copy2(w, psum):
                po, co = (w % 2) * W, (w // 2) * 8
                nc.scalar.copy(out=x2acc[po:po + W, co:co + 8],
                               in_=psum[po:po + W, :])

            def epilogue2(w, psum):
                po, co = (w % 2) * W, (w // 2) * 8
                nc.scalar.mul(out=out_sb[po:po + W, co:co + 8],
                              in_=psum[po:po + W, :],
                              mul=nsd_sb[po:po + W, w:w + 1])

            aggregate(y_full, x2acc, 8, BT2, 8, epilogue2, copy2, bt_o=48)

            out_base = out_d[:]
            nc.sync.dma_start(
                bass.AP(out_base.tensor, out_base.offset,
                        [[8, 128], [128 * 8, NQ], [1, 8]]),
                out_sb[:].rearrange("p (q c) -> p q c", c=8))
    nc.compile()
    return nc


_CACHE = {}
_LAST_RES = None


def kernel(features, src, dst, W1, b1, W2, b2):
    (in_maps, tw, tile_win, tile_base, T, fuse_b1,
     node_at) = _prep(features, src, dst, W1, b1, W2, b2)
    key = (T, fuse_b1, tuple(tw.reshape(-1).tolist()))
    if key not in _CACHE:
        _CACHE[key] = _build_program(tw, tile_win, T, fuse_b1=fuse_b1)
    nc = _CACHE[key]
    res = bass_utils.run_bass_kernel_spmd(
        nc, in_maps, core_ids=list(range(NCORES)))
    global _LAST_RES
    _LAST_RES = res
    out = np.empty((N, C), np.float32)
    b2f = np.asarray(b2, np.float32)
    for c in range(NCORES):
        cslice = node_at[c * NSHP:(c + 1) * NSHP]
        valid = cslice >= 0
        out[cslice[valid]] = res.results[c]["out"][valid][:, :C]
    out += b2f[None, :]
    return out
